# revision 9
# baseline (speedup 1.0000x reference)
"""Block-sparse attention kernel for TRN2 (8 NeuronCores, SPMD).

Math (from the reference nn.Module):
  x [1, 8, 512, 768] -> flatten to [S=4096, 768]
  q/k/v = x @ W{q,k,v}.T, split into H=12 heads of D=64
  block mask: query in view v attends keys [0 : P_v] where
  P_v = 1024 for v in {0,1}, (v+1)*512 for v >= 2  (always a prefix)
  out = softmax(q k^T / 8 + mask) v, merge heads, @ Wo.T + bo

Sharding: core c owns two 256-query groups (group A from a low view,
group B from a high view; see PAIR_A/PAIR_B). Every core computes the
full K/V projections (replicated; avoids collectives) and masks keys
beyond its prefix via an exp-bias of -1e5 per 512-key block.

Dataflow is fully transposed (x^T, K^T, Q^T, out^T) so every matmul has
its contraction dim on partitions and a >=256-wide moving operand
(float32r at full PE rate). Softmax sums come for free from a ones
column interleaved into the V tiles (head h occupies columns h*65..+64,
column h*65+64 is 1.0), so the AV matmul's 65th output row is the
per-query sum of exp.

Engine balance: exp runs on Act (the only engine with activation);
AV-accumulate adds are one [65,256] tensor_add per unit on DVE into a
65-row accumulator (value rows 0..63 + sum row 64 together); V-block
PSUM->SBUF copies run on Pool (gpsimd) to keep DVE under Act/PE; the
score PSUM pool is double-buffered so exp(i) overlaps scores(i+1).
"""

import sys

sys.path.insert(0, "/opt/trn_rl_repo")

import numpy as np

import concourse.bass as bass
import concourse.mybir as mybir
import concourse.tile as tile
from concourse.bass_utils import run_bass_kernel_spmd

F32 = mybir.dt.float32
F32R = mybir.dt.float32r

S, DIM, H, D = 4096, 768, 12, 64
V, L = 8, 512
NC_N = 8
NM = DIM // 128          # 6 chunks of the model dim
NKB = S // 512           # 8 key blocks
SCALE = float(D) ** -0.5
NEG = -1.0e5

# allowed 512-key blocks per view (prefix length / 512)
KB_VIEW = [2, 2, 3, 4, 5, 6, 7, 8]
# core c handles half (c%2) of views PAIR_A[c] (group A, compiled 4 kb)
# and PAIR_B[c] (group B, compiled 8 kb), 256 queries each
PAIR_A = [0, 0, 1, 1, 2, 2, 3, 3]
PAIR_B = [7, 7, 6, 6, 5, 5, 4, 4]
NKB_A = 4
LG = 256  # queries per group


def legalize_multiwaits(nc):
    """This toolchain's walrus accepts at most ONE sync-wait per
    instruction; Tile's sem-assignment happily emits several. Split the
    extras into standalone EventSemaphore (wait) instructions on the same
    engine, placed immediately before the gated instruction."""
    scratch = nc.alloc_semaphore("legalize_scratch")
    fn = nc.m.functions[0]
    for bb in fn.blocks:
        insts = bb.instructions
        out = []
        changed = False
        for inst in insts:
            si = getattr(inst, "sync_info", None)
            ow = list(si.on_wait) if si is not None and si.on_wait else []
            if len(ow) > 1:
                for w in ow[:-1]:
                    ev = nc.engines[inst.engine].nop(nofuse=True)
                    raw = ev.ins
                    raw.sync_info = mybir.SyncInfo(on_wait=[w], on_update=[])
                    # pop it from wherever the builder appended it
                    tail = nc.cur_bb.bb.instructions
                    assert tail[-1].name == raw.name
                    nc.cur_bb.bb.instructions = tail[:-1]
                    out.append(raw)
                si.on_wait = [ow[-1]]
                inst.sync_info = si
                changed = True
            out.append(inst)
        if changed:
            bb.instructions = out


def build_program(nkb=NKB, loop_n=1):
    nc = bass.Bass()
    xT = nc.dram_tensor("xT", [DIM, S], F32, kind="ExternalInput")
    xTq = nc.dram_tensor("xTq", [DIM, L], F32, kind="ExternalInput")
    WqT = nc.dram_tensor("WqT", [DIM, DIM], F32, kind="ExternalInput")
    WkT = nc.dram_tensor("WkT", [DIM, DIM], F32, kind="ExternalInput")
    WvT = nc.dram_tensor("WvT", [DIM, DIM], F32, kind="ExternalInput")
    WoT = nc.dram_tensor("WoT", [DIM, DIM], F32, kind="ExternalInput")
    boT = nc.dram_tensor("boT", [128, NM], F32, kind="ExternalInput")
    maskT = nc.dram_tensor("maskT", [128, NKB_A + NKB], F32, kind="ExternalInput")
    outT = nc.dram_tensor("outT", [DIM, L], F32, kind="ExternalOutput")

    def mm(out, lhsT, rhs, start, stop):
        nc.tensor.matmul(out, lhsT, rhs, start=start, stop=stop)

    with nc.allow_low_precision(reason="f32r accumulators (4-byte)"), \
         tile.TileContext(nc) as tc, \
         tc.tile_pool(name="const", bufs=1) as cpool, \
         tc.tile_pool(name="wres", bufs=1) as wres, \
         tc.tile_pool(name="wstream", bufs=6) as wstr, \
         tc.tile_pool(name="acc", bufs=1) as accp, \
         tc.tile_pool(name="xt", bufs=2) as xtp, \
         tc.tile_pool(name="ktblk", bufs=2) as ktp, \
         tc.tile_pool(name="vblk", bufs=2) as vbp, \
         tc.tile_pool(name="expp", bufs=3) as expp, \
         tc.tile_pool(name="outp", bufs=2) as outp, \
         tc.tile_pool(name="ps_proj", bufs=2, space="PSUM") as psproj, \
         tc.tile_pool(name="ps_sc", bufs=2, space="PSUM") as pssc, \
         tc.tile_pool(name="ps_av", bufs=2, space="PSUM") as psav:

        mask_sb = cpool.tile([128, NKB_A + NKB], F32, name="mask_sb")
        nc.sync.dma_start(mask_sb[:, :], maskT[:, :])
        ones_r = cpool.tile([65, 64], F32R, name="ones_r")
        nc.vector.memset(ones_r[:, :].bitcast(F32), 1.0)
        bo_sb = cpool.tile([128, NM], F32, name="bo_sb")
        nc.sync.dma_start(bo_sb[:, :], boT[:, :])

        # resident K/V weights: block cc at cols cc*DIM, rows = W*T rows
        wk_sb = wres.tile([128, NM * DIM], F32R, name="wk_sb")
        wv_sb = wres.tile([128, NM * DIM], F32R, name="wv_sb")
        for cc in range(NM):
            nc.gpsimd.dma_start(
                wk_sb[:, cc * DIM:(cc + 1) * DIM], WkT[cc * 128:(cc + 1) * 128, :]
            )
            nc.gpsimd.dma_start(
                wv_sb[:, cc * DIM:(cc + 1) * DIM], WvT[cc * 128:(cc + 1) * 128, :]
            )

        for _rep in range(loop_n):
            # ---- Q projection: Q^T[mm-block] = sum_cc WqT[cc,mm].T @ xTq[cc] ----
            # cols [g*NM*LG + mi*LG : +LG] = group g, m-chunk mi
            qt_sb = accp.tile([128, 2 * NM * LG], F32R, name="qt_sb")
            xq_sb = xtp.tile([128, NM * L], F32R, name="xt_t", tag="xt")
            for cc in range(NM):
                nc.gpsimd.dma_start(
                    xq_sb[:, cc * L:(cc + 1) * L], xTq[cc * 128:(cc + 1) * 128, :]
                )
            wq_t = []
            for cc in range(NM):
                w = wstr.tile([128, DIM], F32R, name=f"wq_{cc}", tag="wstr")
                nc.gpsimd.dma_start(w[:, :], WqT[cc * 128:(cc + 1) * 128, :])
                wq_t.append(w)
            for mi in range(NM):
                psq = psproj.tile([128, L], F32, name="psq", tag="proj")
                for cc in range(NM):
                    mm(psq[:, :], wq_t[cc][:, mi * 128:(mi + 1) * 128],
                       xq_sb[:, cc * L:(cc + 1) * L], cc == 0, cc == NM - 1)
                # xTq is [A queries 0:256 | B queries 256:512]
                for g in range(2):
                    nc.vector.tensor_copy(
                        qt_sb[:, g * NM * LG + mi * LG: g * NM * LG + (mi + 1) * LG],
                        psq[:, g * LG:(g + 1) * LG])

            # persistent 65-row accumulator: rows 0..63 = AV values, row 64 =
            # sum of exp. unit (h, g) lives at cols (h*2+g)*LG.
            acc65 = accp.tile([65, 2 * H * LG], F32, name="acc65")

            # ---- key-block loop ----
            for kb in range(nkb):
                xt_b = xtp.tile([128, NM * L], F32R, name="xt_t", tag="xt")
                for cc in range(NM):
                    nc.gpsimd.dma_start(
                        xt_b[:, cc * L:(cc + 1) * L],
                        xT[cc * 128:(cc + 1) * 128, kb * 512:(kb + 1) * 512],
                    )

                # K^T block: [dims(part, by mm), 512 keys]
                kt_b = ktp.tile([128, NM * 512], F32R, name="kt_b", tag="kt")
                for mi in range(NM):
                    psk = psproj.tile([128, 512], F32, name="psk", tag="proj")
                    for cc in range(NM):
                        mm(psk[:, :],
                           wk_sb[:, cc * DIM + mi * 128: cc * DIM + (mi + 1) * 128],
                           xt_b[:, cc * L:(cc + 1) * L], cc == 0, cc == NM - 1)
                    nc.vector.tensor_copy(kt_b[:, mi * 512:(mi + 1) * 512], psk[:, :])

                # V block: 4 sub-chunks of 128 keys; head h at cols h*65..h*65+63,
                # col h*65+64 stays 1.0 from the memset (softmax-sum trick);
                # only those 48 strided columns need the memset
                v_b = vbp.tile([128, 4 * H * (D + 1)], F32R, name="v_b", tag="v")
                ones_cols = v_b[:, :].bitcast(F32).rearrange(
                    "p (x j) -> p x j", j=D + 1)[:, :, D:D + 1]
                nc.vector.memset(ones_cols, 1.0)
                v_sc = [v_b[:, sc * H * (D + 1):(sc + 1) * H * (D + 1)]
                        for sc in range(4)]
                for sc in range(4):
                    v_t = v_sc[sc]
                    for half in range(2):
                        psv = psproj.tile([128, 512], F32, name="psv", tag="proj")
                        for cc in range(NM):
                            mm(psv[:, 0:384],
                               xt_b[:, cc * L + sc * 128: cc * L + (sc + 1) * 128],
                               wv_sb[:, cc * DIM + half * 384: cc * DIM + (half + 1) * 384],
                               cc == 0, cc == NM - 1)
                        dst = v_t[:, half * 6 * 65:(half + 1) * 6 * 65]
                        dst = dst.rearrange("p (h j) -> p h j", j=65)[:, :, 0:64]
                        srcp = psv[:, 0:384].rearrange("p (h j) -> p h j", j=64)
                        # split the PSUM->SBUF copies between Act and DVE;
                        # exp/copy share one act table so no reload thrash
                        if sc < 2:
                            nc.scalar.copy(dst, srcp)
                        else:
                            nc.vector.tensor_copy(dst, srcp)

                # attention for all heads against this key block; group A only
                # participates for kb < NKB_A (its compiled prefix)
                for h in range(H):
                    po = (h % 2) * 64            # partition offset of head h
                    co = (h // 2) * 512          # col offset (mm block h//2)
                    groups = [1] if kb >= NKB_A else [1, 0]
                    for g in groups:
                        qcol = g * NM * LG + (h // 2) * LG
                        mcol = kb if g == 0 else NKB_A + kb
                        ps_s = pssc.tile([128, 1024], F32, name="ps_s", tag="sc")
                        for sc in range(4):
                            mm(ps_s[:, sc * LG:(sc + 1) * LG],
                               kt_b[po:po + 64, co + sc * 128: co + (sc + 1) * 128],
                               qt_sb[po:po + 64, qcol:qcol + LG], True, True)
                        exp_t = expp.tile([128, 1024], F32R, name="exp_t", tag="exp")
                        nc.scalar.activation(
                            exp_t[:, :], ps_s[:, :],
                            mybir.ActivationFunctionType.Exp,
                            bias=mask_sb[:, mcol:mcol + 1], scale=SCALE,
                        )
                        ps_o = psav.tile([D + 1, LG], F32, name="ps_o", tag="av")
                        for sc in range(4):
                            mm(ps_o[:, :], v_sc[sc][:, h * 65:(h + 1) * 65],
                               exp_t[:, sc * LG:(sc + 1) * LG], sc == 0, sc == 3)
                        ucol = (h * 2 + g) * LG
                        if kb == 0:
                            nc.vector.tensor_copy(acc65[:, ucol:ucol + LG],
                                                  ps_o[:, :])
                        else:
                            nc.vector.tensor_add(acc65[:, ucol:ucol + LG],
                                                 acc65[:, ucol:ucol + LG],
                                                 ps_o[:, :])

            # ---- normalize + output projection ----
            wo_t = []
            for cc in range(NM):
                w = wstr.tile([128, DIM], F32R, name=f"wo_{cc}", tag="wstr")
                nc.gpsimd.dma_start(w[:, :], WoT[cc * 128:(cc + 1) * 128, :])
                wo_t.append(w)

            # at_acc: O-proj moving operand layout [128 (2 heads), NM*LG] per
            # group; built by fused repack+normalize from acc65
            at_acc = accp.tile([128, 2 * NM * LG], F32R, name="at_acc")
            for g in range(2):
                for mmi in range(NM):
                    # broadcast 1/sum across partitions via outer product with
                    # ones; the two heads land in different PSUM banks
                    rb_ps = pssc.tile([128, 1024], F32, name="rb_ps", tag="sc")
                    for sub in range(2):
                        h = mmi * 2 + sub
                        ucol = (h * 2 + g) * LG
                        # 1/sum staged to partition 0 as f32r for the
                        # ones-outer-product broadcast matmul
                        stage = outp.tile([1, LG], F32R, name="stage_r",
                                          tag="stg")
                        nc.vector.reciprocal(stage[0:1, :],
                                             acc65[64:65, ucol:ucol + LG])
                        mm(rb_ps[0:64, sub * 512:sub * 512 + LG],
                           ones_r[0:1, :], stage[0:1, :], True, True)
                    acol = g * NM * LG + mmi * LG
                    for sub in range(2):
                        h = mmi * 2 + sub
                        ucol = (h * 2 + g) * LG
                        nc.vector.tensor_mul(
                            at_acc[sub * 64:(sub + 1) * 64, acol:acol + LG],
                            acc65[0:64, ucol:ucol + LG],
                            rb_ps[0:64, sub * 512:sub * 512 + LG])
            for g in range(2):
                for mmi in range(NM):
                    pso = psproj.tile([128, L], F32, name="pso", tag="proj")
                    for cc in range(NM):
                        mm(pso[:, 0:LG], wo_t[cc][:, mmi * 128:(mmi + 1) * 128],
                           at_acc[:, g * NM * LG + cc * LG: g * NM * LG + (cc + 1) * LG],
                           cc == 0, cc == NM - 1)
                    out_t = outp.tile([128, LG], F32, name="out_t", tag="out")
                    nc.vector.tensor_scalar_add(out_t[:, :], pso[:, 0:LG],
                                                bo_sb[:, mmi:mmi + 1])
                    nc.sync.dma_start(
                        outT[mmi * 128:(mmi + 1) * 128, g * LG:(g + 1) * LG],
                        out_t[:, :])

    legalize_multiwaits(nc)
    return nc


_program = None


def make_in_maps(x, Wq, Wk, Wv, Wo, bo):
    xf = np.ascontiguousarray(np.asarray(x, np.float32).reshape(S, DIM))
    xT = np.ascontiguousarray(xf.T)
    shared = {
        "xT": xT,
        "WqT": np.ascontiguousarray(np.asarray(Wq, np.float32).T),
        "WkT": np.ascontiguousarray(np.asarray(Wk, np.float32).T),
        "WvT": np.ascontiguousarray(np.asarray(Wv, np.float32).T),
        "WoT": np.ascontiguousarray(np.asarray(Wo, np.float32).T),
        "boT": np.ascontiguousarray(
            np.asarray(bo, np.float32).reshape(NM, 128).T
        ),
    }
    in_maps = []
    for c in range(NC_N):
        hf = c % 2
        va, vb = PAIR_A[c], PAIR_B[c]
        ra = va * 512 + hf * LG
        rb = vb * 512 + hf * LG
        mask = np.zeros((128, NKB_A + NKB), np.float32)
        mask[:, KB_VIEW[va]:NKB_A] = NEG
        mask[:, NKB_A + KB_VIEW[vb]:] = NEG
        m = dict(shared)
        m["xTq"] = np.ascontiguousarray(
            np.concatenate([xT[:, ra:ra + LG], xT[:, rb:rb + LG]], axis=1))
        m["maskT"] = mask
        in_maps.append(m)
    return in_maps


def kernel(x, Wq, Wk, Wv, Wo, bo):
    global _program
    in_maps = make_in_maps(x, Wq, Wk, Wv, Wo, bo)
    if _program is None:
        _program = build_program()
    ret = run_bass_kernel_spmd(_program, in_maps, list(range(NC_N)))
    out = np.empty((S, DIM), np.float32)
    for c in range(NC_N):
        hf = c % 2
        oT = ret.results[c]["outT"]
        ra = PAIR_A[c] * 512 + hf * LG
        rb = PAIR_B[c] * 512 + hf * LG
        out[ra:ra + LG, :] = oT[:, 0:LG].T
        out[rb:rb + LG, :] = oT[:, LG:2 * LG].T
    return out.reshape(1, V, L, DIM)


# revision 10
# speedup vs baseline: 1.1410x; 1.1410x over previous
"""Block-sparse attention kernel for TRN2 (8 NeuronCores, SPMD).

Math (from the reference nn.Module):
  x [1, 8, 512, 768] -> flatten to [S=4096, 768]
  q/k/v = x @ W{q,k,v}.T, split into H=12 heads of D=64
  block mask: query in view v attends keys [0 : P_v] where
  P_v = 1024 for v in {0,1}, (v+1)*512 for v >= 2  (always a prefix)
  out = softmax(q k^T / 8 + mask) v, merge heads, @ Wo.T + bo

Sharding: core c owns two 256-query groups (group A from a low view,
group B from a high view; see PAIR_A/PAIR_B). Every core computes the
full K/V projections (replicated; avoids collectives) and masks keys
beyond its prefix via an exp-bias of -1e5 per 512-key block.

Dataflow is fully transposed (x^T, K^T, Q^T, out^T) so every matmul has
its contraction dim on partitions and a >=256-wide moving operand
(float32r at full PE rate). Softmax sums come for free from a ones
column interleaved into the V tiles (head h occupies columns h*65..+64,
column h*65+64 is 1.0), so the AV matmul's 65th output row is the
per-query sum of exp.

Engine balance: exp runs on Act (the only engine with activation);
AV-accumulate adds are one [65,256] tensor_add per unit on DVE into a
65-row accumulator (value rows 0..63 + sum row 64 together); V-block
PSUM->SBUF copies run on Pool (gpsimd) to keep DVE under Act/PE; the
score PSUM pool is double-buffered so exp(i) overlaps scores(i+1).
"""

import sys

sys.path.insert(0, "/opt/trn_rl_repo")

import numpy as np

import concourse.bass as bass
import concourse.mybir as mybir
import concourse.tile as tile
from concourse.bass_utils import run_bass_kernel_spmd

F32 = mybir.dt.float32
F32R = mybir.dt.float32r

S, DIM, H, D = 4096, 768, 12, 64
V, L = 8, 512
NC_N = 8
NM = DIM // 128          # 6 chunks of the model dim
NKB = S // 512           # 8 key blocks
SCALE = float(D) ** -0.5
NEG = -1.0e5

# allowed 512-key blocks per view (prefix length / 512)
KB_VIEW = [2, 2, 3, 4, 5, 6, 7, 8]
# core c handles half (c%2) of views PAIR_A[c] (group A, compiled 4 kb)
# and PAIR_B[c] (group B, compiled 8 kb), 256 queries each
PAIR_A = [0, 0, 1, 1, 2, 2, 3, 3]
PAIR_B = [7, 7, 6, 6, 5, 5, 4, 4]
NKB_A = 4
LG = 256  # queries per group


def legalize_multiwaits(nc):
    """This toolchain's walrus accepts at most ONE sync-wait per
    instruction; Tile's sem-assignment happily emits several. Split the
    extras into standalone EventSemaphore (wait) instructions on the same
    engine, placed immediately before the gated instruction."""
    scratch = nc.alloc_semaphore("legalize_scratch")
    fn = nc.m.functions[0]
    for bb in fn.blocks:
        insts = bb.instructions
        out = []
        changed = False
        for inst in insts:
            si = getattr(inst, "sync_info", None)
            ow = list(si.on_wait) if si is not None and si.on_wait else []
            if len(ow) > 1:
                for w in ow[:-1]:
                    ev = nc.engines[inst.engine].nop(nofuse=True)
                    raw = ev.ins
                    raw.sync_info = mybir.SyncInfo(on_wait=[w], on_update=[])
                    # pop it from wherever the builder appended it
                    tail = nc.cur_bb.bb.instructions
                    assert tail[-1].name == raw.name
                    nc.cur_bb.bb.instructions = tail[:-1]
                    out.append(raw)
                si.on_wait = [ow[-1]]
                inst.sync_info = si
                changed = True
            out.append(inst)
        if changed:
            bb.instructions = out


def build_program(nkb=NKB, loop_n=1):
    nc = bass.Bass()
    xT = nc.dram_tensor("xT", [DIM, S], F32, kind="ExternalInput")
    xTq = nc.dram_tensor("xTq", [DIM, L], F32, kind="ExternalInput")
    WqT = nc.dram_tensor("WqT", [DIM, DIM], F32, kind="ExternalInput")
    WkT = nc.dram_tensor("WkT", [DIM, DIM], F32, kind="ExternalInput")
    WvT = nc.dram_tensor("WvT", [DIM, DIM], F32, kind="ExternalInput")
    WoT = nc.dram_tensor("WoT", [DIM, DIM], F32, kind="ExternalInput")
    boT = nc.dram_tensor("boT", [128, NM], F32, kind="ExternalInput")
    maskT = nc.dram_tensor("maskT", [128, NKB_A + NKB], F32, kind="ExternalInput")
    outT = nc.dram_tensor("outT", [DIM, L], F32, kind="ExternalOutput")

    def mm(out, lhsT, rhs, start, stop):
        nc.tensor.matmul(out, lhsT, rhs, start=start, stop=stop)

    with nc.allow_low_precision(reason="f32r accumulators (4-byte)"), \
         tile.TileContext(nc) as tc, \
         tc.tile_pool(name="const", bufs=1) as cpool, \
         tc.tile_pool(name="wres", bufs=1) as wres, \
         tc.tile_pool(name="wstream", bufs=6) as wstr, \
         tc.tile_pool(name="acc", bufs=1) as accp, \
         tc.tile_pool(name="xt", bufs=2) as xtp, \
         tc.tile_pool(name="ktblk", bufs=2) as ktp, \
         tc.tile_pool(name="vblk", bufs=2) as vbp, \
         tc.tile_pool(name="expp", bufs=3) as expp, \
         tc.tile_pool(name="outp", bufs=2) as outp, \
         tc.tile_pool(name="ps_proj", bufs=2, space="PSUM") as psproj, \
         tc.tile_pool(name="ps_sc", bufs=2, space="PSUM") as pssc, \
         tc.tile_pool(name="ps_av", bufs=2, space="PSUM") as psav:

        mask_sb = cpool.tile([128, NKB_A + NKB], F32, name="mask_sb")
        nc.sync.dma_start(mask_sb[:, :], maskT[:, :])
        ones_r = cpool.tile([65, 64], F32R, name="ones_r")
        nc.vector.memset(ones_r[:, :].bitcast(F32), 1.0)
        bo_sb = cpool.tile([128, NM], F32, name="bo_sb")
        nc.sync.dma_start(bo_sb[:, :], boT[:, :])

        # resident K/V weights: block cc at cols cc*DIM, rows = W*T rows
        wk_sb = wres.tile([128, NM * DIM], F32R, name="wk_sb")
        wv_sb = wres.tile([128, NM * DIM], F32R, name="wv_sb")
        for cc in range(NM):
            nc.gpsimd.dma_start(
                wk_sb[:, cc * DIM:(cc + 1) * DIM], WkT[cc * 128:(cc + 1) * 128, :]
            )
            nc.gpsimd.dma_start(
                wv_sb[:, cc * DIM:(cc + 1) * DIM], WvT[cc * 128:(cc + 1) * 128, :]
            )

        for _rep in range(loop_n):
            # ---- Q projection: Q^T[mm-block] = sum_cc WqT[cc,mm].T @ xTq[cc] ----
            # cols [g*NM*LG + mi*LG : +LG] = group g, m-chunk mi
            qt_sb = accp.tile([128, 2 * NM * LG], F32R, name="qt_sb")
            xq_sb = xtp.tile([128, NM * L], F32R, name="xt_t", tag="xt")
            for cc in range(NM):
                nc.gpsimd.dma_start(
                    xq_sb[:, cc * L:(cc + 1) * L], xTq[cc * 128:(cc + 1) * 128, :]
                )
            wq_t = []
            for cc in range(NM):
                w = wstr.tile([128, DIM], F32R, name=f"wq_{cc}", tag="wstr")
                nc.gpsimd.dma_start(w[:, :], WqT[cc * 128:(cc + 1) * 128, :])
                wq_t.append(w)
            for mi in range(NM):
                psq = psproj.tile([128, L], F32, name="psq", tag="proj")
                for cc in range(NM):
                    mm(psq[:, :], wq_t[cc][:, mi * 128:(mi + 1) * 128],
                       xq_sb[:, cc * L:(cc + 1) * L], cc == 0, cc == NM - 1)
                # xTq is [A queries 0:256 | B queries 256:512]
                for g in range(2):
                    nc.vector.tensor_copy(
                        qt_sb[:, g * NM * LG + mi * LG: g * NM * LG + (mi + 1) * LG],
                        psq[:, g * LG:(g + 1) * LG])

            # persistent 65-row accumulator: rows 0..63 = AV values, row 64 =
            # sum of exp. unit (h, g) lives at cols (h*2+g)*LG.
            acc65 = accp.tile([65, 2 * H * LG], F32, name="acc65")
            # at_acc: O-proj moving operand layout [128 (2 heads), NM*LG] per
            # group; built by fused repack+normalize from acc65
            at_acc = accp.tile([128, 2 * NM * LG], F32R, name="at_acc")

            # per-kb tile state for the software pipeline
            xt_tiles, kt_tiles, vsc_tiles = {}, {}, {}
            wo_t = []

            def dma_xt(kb):
                xt_b = xtp.tile([128, NM * L], F32R, name="xt_t", tag="xt")
                for cc in range(NM):
                    nc.gpsimd.dma_start(
                        xt_b[:, cc * L:(cc + 1) * L],
                        xT[cc * 128:(cc + 1) * 128, kb * 512:(kb + 1) * 512],
                    )
                xt_tiles[kb] = xt_b

            def proj_steps(kb):
                # 14 closures: 6 K^T m-chunks + 8 V (sc, half) chunks,
                # each one PSUM group — interleaved between attention units
                # of the previous block so PE never idles
                def psk_step(mi, kb=kb):
                    xt_b = xt_tiles[kb]
                    if mi == 0:
                        kt_tiles[kb] = ktp.tile([128, NM * 512], F32R,
                                                name="kt_b", tag="kt")
                    kt_b = kt_tiles[kb]
                    psk = psproj.tile([128, 512], F32, name="psk", tag="proj")
                    for cc in range(NM):
                        mm(psk[:, :],
                           wk_sb[:, cc * DIM + mi * 128: cc * DIM + (mi + 1) * 128],
                           xt_b[:, cc * L:(cc + 1) * L], cc == 0, cc == NM - 1)
                    nc.vector.tensor_copy(kt_b[:, mi * 512:(mi + 1) * 512],
                                          psk[:, :])

                def psv_step(sc, half, kb=kb):
                    xt_b = xt_tiles[kb]
                    if sc == 0 and half == 0:
                        v_b = vbp.tile([128, 4 * H * (D + 1)], F32R,
                                       name="v_b", tag="v")
                        # head h at cols h*65..h*65+63 of each sub-chunk,
                        # col h*65+64 = 1.0 (softmax-sum trick): memset only
                        # those 48 strided columns
                        ones_cols = v_b[:, :].bitcast(F32).rearrange(
                            "p (x j) -> p x j", j=D + 1)[:, :, D:D + 1]
                        nc.vector.memset(ones_cols, 1.0)
                        vsc_tiles[kb] = [
                            v_b[:, s * H * (D + 1):(s + 1) * H * (D + 1)]
                            for s in range(4)]
                    v_t = vsc_tiles[kb][sc]
                    psv = psproj.tile([128, 512], F32, name="psv", tag="proj")
                    for cc in range(NM):
                        mm(psv[:, 0:384],
                           xt_b[:, cc * L + sc * 128: cc * L + (sc + 1) * 128],
                           wv_sb[:, cc * DIM + half * 384: cc * DIM + (half + 1) * 384],
                           cc == 0, cc == NM - 1)
                    dst = v_t[:, half * 6 * 65:(half + 1) * 6 * 65]
                    dst = dst.rearrange("p (h j) -> p h j", j=65)[:, :, 0:64]
                    srcp = psv[:, 0:384].rearrange("p (h j) -> p h j", j=64)
                    # split the PSUM->SBUF copies between Act and DVE;
                    # exp/copy share one act table so no reload thrash
                    if sc < 2:
                        nc.scalar.copy(dst, srcp)
                    else:
                        nc.vector.tensor_copy(dst, srcp)

                steps = [lambda mi=mi: psk_step(mi) for mi in range(NM)]
                steps += [lambda sc=sc, half=half: psv_step(sc, half)
                          for sc in range(4) for half in range(2)]
                return steps

            def attn_unit(kb, h, g):
                po = (h % 2) * 64            # partition offset of head h
                co = (h // 2) * 512          # col offset (mm block h//2)
                kt_b = kt_tiles[kb]
                qcol = g * NM * LG + (h // 2) * LG
                mcol = kb if g == 0 else NKB_A + kb
                ps_s = pssc.tile([128, 1024], F32, name="ps_s", tag="sc")
                for sc in range(4):
                    mm(ps_s[:, sc * LG:(sc + 1) * LG],
                       kt_b[po:po + 64, co + sc * 128: co + (sc + 1) * 128],
                       qt_sb[po:po + 64, qcol:qcol + LG], True, True)
                exp_t = expp.tile([128, 1024], F32R, name="exp_t", tag="exp")
                nc.scalar.activation(
                    exp_t[:, :], ps_s[:, :],
                    mybir.ActivationFunctionType.Exp,
                    bias=mask_sb[:, mcol:mcol + 1], scale=SCALE,
                )
                ps_o = psav.tile([D + 1, LG], F32, name="ps_o", tag="av")
                for sc in range(4):
                    mm(ps_o[:, :], vsc_tiles[kb][sc][:, h * 65:(h + 1) * 65],
                       exp_t[:, sc * LG:(sc + 1) * LG], sc == 0, sc == 3)
                ucol = (h * 2 + g) * LG
                if kb == 0:
                    nc.vector.tensor_copy(acc65[:, ucol:ucol + LG], ps_o[:, :])
                else:
                    nc.vector.tensor_add(acc65[:, ucol:ucol + LG],
                                         acc65[:, ucol:ucol + LG], ps_o[:, :])

            def norm_step(g, mmi):
                # broadcast 1/sum across partitions via outer product with
                # ones; the two heads land in different PSUM banks, then the
                # normalize multiply repacks acc65 into the O-proj layout
                rb_ps = pssc.tile([128, 1024], F32, name="rb_ps", tag="sc")
                for sub in range(2):
                    h = mmi * 2 + sub
                    ucol = (h * 2 + g) * LG
                    stage = outp.tile([1, LG], F32R, name="stage_r", tag="stg")
                    nc.vector.reciprocal(stage[0:1, :],
                                         acc65[64:65, ucol:ucol + LG])
                    mm(rb_ps[0:64, sub * 512:sub * 512 + LG],
                       ones_r[0:1, :], stage[0:1, :], True, True)
                acol = g * NM * LG + mmi * LG
                for sub in range(2):
                    h = mmi * 2 + sub
                    ucol = (h * 2 + g) * LG
                    nc.vector.tensor_mul(
                        at_acc[sub * 64:(sub + 1) * 64, acol:acol + LG],
                        acc65[0:64, ucol:ucol + LG],
                        rb_ps[0:64, sub * 512:sub * 512 + LG])

            def oproj_step(g, mmi):
                pso = psproj.tile([128, L], F32, name="pso", tag="proj")
                for cc in range(NM):
                    mm(pso[:, 0:LG], wo_t[cc][:, mmi * 128:(mmi + 1) * 128],
                       at_acc[:, g * NM * LG + cc * LG: g * NM * LG + (cc + 1) * LG],
                       cc == 0, cc == NM - 1)
                out_t = outp.tile([128, LG], F32, name="out_t", tag="out")
                nc.vector.tensor_scalar_add(out_t[:, :], pso[:, 0:LG],
                                            bo_sb[:, mmi:mmi + 1])
                nc.sync.dma_start(
                    outT[mmi * 128:(mmi + 1) * 128, g * LG:(g + 1) * LG],
                    out_t[:, :])

            def wo_dma_step(cc):
                w = wstr.tile([128, DIM], F32R, name=f"wo_{cc}", tag="wstr")
                nc.gpsimd.dma_start(w[:, :], WoT[cc * 128:(cc + 1) * 128, :])
                wo_t.append(w)

            # ---- prologue: first block projected up front ----
            dma_xt(0)
            for st in proj_steps(0):
                st()
            dma_xt(1)

            # ---- key-block loop, software-pipelined ----
            for kb in range(nkb):
                units = [(h, g) for h in range(H)
                         for g in ([1] if kb >= NKB_A else [1, 0])]
                steps = []
                if kb + 2 < nkb:
                    steps.append(lambda kb=kb: dma_xt(kb + 2))
                if kb + 1 < nkb:
                    steps += proj_steps(kb + 1)
                if kb == NKB_A - 1:
                    steps += [lambda cc=cc: wo_dma_step(cc)
                              for cc in range(NM)]
                if kb == NKB_A:
                    # group A (g=0) is final after kb 3: normalize it here
                    steps += [lambda mmi=mmi: norm_step(0, mmi)
                              for mmi in range(NM)]
                if kb == NKB_A + 1:
                    steps += [lambda mmi=mmi: oproj_step(0, mmi)
                              for mmi in range(NM)]
                nu, ns = len(units), len(steps)
                si = 0
                for i, (h, g) in enumerate(units):
                    attn_unit(kb, h, g)
                    target = (i + 1) * ns // nu
                    while si < target:
                        steps[si]()
                        si += 1

            # ---- tail: group B normalize + output projection ----
            for mmi in range(NM):
                norm_step(1, mmi)
            for mmi in range(NM):
                oproj_step(1, mmi)

    legalize_multiwaits(nc)
    return nc


_program = None


def make_in_maps(x, Wq, Wk, Wv, Wo, bo):
    xf = np.ascontiguousarray(np.asarray(x, np.float32).reshape(S, DIM))
    xT = np.ascontiguousarray(xf.T)
    shared = {
        "xT": xT,
        "WqT": np.ascontiguousarray(np.asarray(Wq, np.float32).T),
        "WkT": np.ascontiguousarray(np.asarray(Wk, np.float32).T),
        "WvT": np.ascontiguousarray(np.asarray(Wv, np.float32).T),
        "WoT": np.ascontiguousarray(np.asarray(Wo, np.float32).T),
        "boT": np.ascontiguousarray(
            np.asarray(bo, np.float32).reshape(NM, 128).T
        ),
    }
    in_maps = []
    for c in range(NC_N):
        hf = c % 2
        va, vb = PAIR_A[c], PAIR_B[c]
        ra = va * 512 + hf * LG
        rb = vb * 512 + hf * LG
        mask = np.zeros((128, NKB_A + NKB), np.float32)
        mask[:, KB_VIEW[va]:NKB_A] = NEG
        mask[:, NKB_A + KB_VIEW[vb]:] = NEG
        m = dict(shared)
        m["xTq"] = np.ascontiguousarray(
            np.concatenate([xT[:, ra:ra + LG], xT[:, rb:rb + LG]], axis=1))
        m["maskT"] = mask
        in_maps.append(m)
    return in_maps


def kernel(x, Wq, Wk, Wv, Wo, bo):
    global _program
    in_maps = make_in_maps(x, Wq, Wk, Wv, Wo, bo)
    if _program is None:
        _program = build_program()
    ret = run_bass_kernel_spmd(_program, in_maps, list(range(NC_N)))
    out = np.empty((S, DIM), np.float32)
    for c in range(NC_N):
        hf = c % 2
        oT = ret.results[c]["outT"]
        ra = PAIR_A[c] * 512 + hf * LG
        rb = PAIR_B[c] * 512 + hf * LG
        out[ra:ra + LG, :] = oT[:, 0:LG].T
        out[rb:rb + LG, :] = oT[:, LG:2 * LG].T
    return out.reshape(1, V, L, DIM)


# revision 18
# speedup vs baseline: 201.8438x; 176.8954x over previous
"""Block-sparse attention kernel for TRN2 (8 NeuronCores, SPMD).

Math (from the reference nn.Module):
  x [1, 8, 512, 768] -> flatten to [S=4096, 768]
  q/k/v = x @ W{q,k,v}.T, split into H=12 heads of D=64
  block mask: query in view v attends keys [0 : P_v] where
  P_v = 1024 for v in {0,1}, (v+1)*512 for v >= 2  (always a prefix)
  out = softmax(q k^T / 8 + mask) v, merge heads, @ Wo.T + bo

Sharding: core c owns two 256-query groups (group A from a low view,
group B from a high view; see PAIR_A/PAIR_B). Every core computes the
full K/V projections (replicated; avoids collectives) and masks keys
beyond its prefix via an exp-bias of -1e5 per 512-key block.

Dataflow is fully transposed (x^T, K^T, Q^T, out^T) so every matmul has
its contraction dim on partitions; projections run f32r with a >=256
moving operand (full PE rate), attention (Q^T, K^T, V, exp) runs bf16
(also full rate, ~4e-3 rel err, well inside the 2e-2 budget). Softmax
sums come free from a ones column interleaved into the V tiles (head h
at columns h*65..+64, column h*65+64 is 1.0): the AV matmul's 65th
output row is the per-query sum of exp.

Schedule: key blocks are processed in PAIRS. Per (head, group)
unit-pair, the two AV matmul groups accumulate into one [65, 256] PSUM
tile (values + sum row together), so only ONE DVE add per unit-pair
lands in the 65-row accumulator. The K/V projection of the next pair
plus the group-A normalize/output-projection (final after kb 3) are
interleaved between attention units so the PE never idles (TRN2's
p-state ramp halves PE clock after every stall). exp runs on Act (the
only engine with activation); V-block PSUM->SBUF copies are split
Act/DVE; the score PSUM pool is double-buffered so exp overlaps the
next scores matmul.
"""

import sys

sys.path.insert(0, "/opt/trn_rl_repo")

import numpy as np

import concourse.bass as bass
import concourse.mybir as mybir
import concourse.tile as tile
from concourse.bass_utils import run_bass_kernel_spmd

F32 = mybir.dt.float32
F32R = mybir.dt.float32r
BF16 = mybir.dt.bfloat16

S, DIM, H, D = 4096, 768, 12, 64
V, L = 8, 512
NC_N = 8
NM = DIM // 128          # 6 chunks of the model dim
NKB = S // 512           # 8 key blocks
SCALE = float(D) ** -0.5
NEG = -1.0e5

# allowed 512-key blocks per view (prefix length / 512)
KB_VIEW = [2, 2, 3, 4, 5, 6, 7, 8]
# core c handles half (c%2) of views PAIR_A[c] (group A, compiled 4 kb)
# and PAIR_B[c] (group B, compiled 8 kb), 256 queries each
PAIR_A = [0, 0, 1, 1, 2, 2, 3, 3]
PAIR_B = [7, 7, 6, 6, 5, 5, 4, 4]
NKB_A = 4
LG = 256  # queries per group


def legalize_multiwaits(nc):
    """This toolchain's walrus accepts at most ONE sync-wait per
    instruction; Tile's sem-assignment happily emits several. Split the
    extras into standalone EventSemaphore (wait) instructions on the same
    engine, placed immediately before the gated instruction."""
    scratch = nc.alloc_semaphore("legalize_scratch")
    fn = nc.m.functions[0]
    for bb in fn.blocks:
        insts = bb.instructions
        out = []
        changed = False
        for inst in insts:
            si = getattr(inst, "sync_info", None)
            ow = list(si.on_wait) if si is not None and si.on_wait else []
            if len(ow) > 1:
                for w in ow[:-1]:
                    ev = nc.engines[inst.engine].nop(nofuse=True)
                    raw = ev.ins
                    raw.sync_info = mybir.SyncInfo(on_wait=[w], on_update=[])
                    # pop it from wherever the builder appended it
                    tail = nc.cur_bb.bb.instructions
                    assert tail[-1].name == raw.name
                    nc.cur_bb.bb.instructions = tail[:-1]
                    out.append(raw)
                si.on_wait = [ow[-1]]
                inst.sync_info = si
                changed = True
            out.append(inst)
        if changed:
            bb.instructions = out


def build_program(nkb=NKB, loop_n=1):
    assert nkb % 2 == 0, "key blocks are processed in pairs"
    nc = bass.Bass()
    xT = nc.dram_tensor("xT", [DIM, S], BF16, kind="ExternalInput")
    xTq = nc.dram_tensor("xTq", [DIM, L], BF16, kind="ExternalInput")
    WqT = nc.dram_tensor("WqT", [DIM, DIM], BF16, kind="ExternalInput")
    WkT = nc.dram_tensor("WkT", [DIM, DIM], BF16, kind="ExternalInput")
    WvT = nc.dram_tensor("WvT", [DIM, DIM], BF16, kind="ExternalInput")
    WoT = nc.dram_tensor("WoT", [DIM, DIM], BF16, kind="ExternalInput")
    boT = nc.dram_tensor("boT", [128, NM], F32, kind="ExternalInput")
    maskT = nc.dram_tensor("maskT", [128, NKB_A + NKB], F32, kind="ExternalInput")
    outT = nc.dram_tensor("outT", [DIM, L], F32, kind="ExternalOutput")

    def mm(out, lhsT, rhs, start, stop):
        nc.tensor.matmul(out, lhsT, rhs, start=start, stop=stop)

    with nc.allow_low_precision(reason="bf16 attention operands, ~4e-3"), \
         tile.TileContext(nc) as tc, \
         tc.tile_pool(name="const", bufs=1) as cpool, \
         tc.tile_pool(name="wres", bufs=1) as wres, \
         tc.tile_pool(name="wstream", bufs=6) as wstr, \
         tc.tile_pool(name="acc", bufs=1) as accp, \
         tc.tile_pool(name="xt", bufs=3) as xtp, \
         tc.tile_pool(name="ktblk", bufs=4) as ktp, \
         tc.tile_pool(name="vblk", bufs=4) as vbp, \
         tc.tile_pool(name="expp", bufs=4) as expp, \
         tc.tile_pool(name="outp", bufs=2) as outp, \
         tc.tile_pool(name="ps_proj", bufs=2, space="PSUM") as psproj, \
         tc.tile_pool(name="ps_sc", bufs=2, space="PSUM") as pssc, \
         tc.tile_pool(name="ps_av", bufs=2, space="PSUM") as psav:

        mask_sb = cpool.tile([128, NKB_A + NKB], F32, name="mask_sb")
        nc.sync.dma_start(mask_sb[:, :], maskT[:, :])
        ones_r = cpool.tile([65, 64], F32R, name="ones_r")
        nc.vector.memset(ones_r[:, :].bitcast(F32), 1.0)
        bo_sb = cpool.tile([128, NM], F32, name="bo_sb")
        nc.sync.dma_start(bo_sb[:, :], boT[:, :])

        # resident K/V weights: block cc at cols cc*DIM, rows = W*T rows
        wk_sb = wres.tile([128, NM * DIM], BF16, name="wk_sb")
        wv_sb = wres.tile([128, NM * DIM], BF16, name="wv_sb")
        # SP ring: keeps the bulk K/V weight load off the gpsimd ring so
        # the Q-projection operands (xq, wq) land first
        for cc in range(NM):
            nc.sync.dma_start(
                wk_sb[:, cc * DIM:(cc + 1) * DIM], WkT[cc * 128:(cc + 1) * 128, :]
            )
            nc.sync.dma_start(
                wv_sb[:, cc * DIM:(cc + 1) * DIM], WvT[cc * 128:(cc + 1) * 128, :]
            )

        for _rep in range(loop_n):
            # ---- Q projection: Q^T[mm-block] = sum_cc WqT[cc,mm].T @ xTq[cc] ----
            # cols [g*NM*LG + mi*LG : +LG] = group g, m-chunk mi
            qt_sb = accp.tile([128, 2 * NM * LG], BF16, name="qt_sb")
            xq_sb = xtp.tile([128, NM * L], BF16, name="xt_t", tag="xt")
            for cc in range(NM):
                nc.gpsimd.dma_start(
                    xq_sb[:, cc * L:(cc + 1) * L], xTq[cc * 128:(cc + 1) * 128, :]
                )
            wq_t = []
            for cc in range(NM):
                w = wstr.tile([128, DIM], BF16, name=f"wq_{cc}", tag="wstr")
                nc.gpsimd.dma_start(w[:, :], WqT[cc * 128:(cc + 1) * 128, :])
                wq_t.append(w)
            for mi in range(NM):
                psq = psproj.tile([128, L], F32, name="psq", tag="proj")
                for cc in range(NM):
                    mm(psq[:, :], wq_t[cc][:, mi * 128:(mi + 1) * 128],
                       xq_sb[:, cc * L:(cc + 1) * L], cc == 0, cc == NM - 1)
                # xTq is [A queries 0:256 | B queries 256:512]
                for g in range(2):
                    nc.vector.tensor_copy(
                        qt_sb[:, g * NM * LG + mi * LG: g * NM * LG + (mi + 1) * LG],
                        psq[:, g * LG:(g + 1) * LG])

            # persistent 65-row accumulator: rows 0..63 = AV values, row 64 =
            # sum of exp. unit (h, g) lives at cols (h*2+g)*LG.
            acc65 = accp.tile([65, 2 * H * LG], F32, name="acc65")
            # at_acc: O-proj moving operand layout [128 (2 heads), NM*LG] per
            # group; built by fused repack+normalize from acc65
            at_acc = accp.tile([128, 2 * NM * LG], BF16, name="at_acc")

            # per-kb tile state for the software pipeline
            xt_tiles, kt_tiles, vsc_tiles = {}, {}, {}
            wo_t = []

            def dma_xt(kb):
                xt_b = xtp.tile([128, NM * L], BF16, name="xt_t", tag="xt")
                for cc in range(NM):
                    nc.gpsimd.dma_start(
                        xt_b[:, cc * L:(cc + 1) * L],
                        xT[cc * 128:(cc + 1) * 128, kb * 512:(kb + 1) * 512],
                    )
                xt_tiles[kb] = xt_b

            def proj_steps(kb):
                # 14 closures: 6 K^T m-chunks + 8 V (sc, half) chunks,
                # each one PSUM group — interleaved between attention units
                # of the previous pair so PE never idles
                def psk_step(mi, kb=kb):
                    xt_b = xt_tiles[kb]
                    if mi == 0:
                        kt_tiles[kb] = ktp.tile([128, NM * 512], BF16,
                                                name="kt_b", tag="kt")
                    kt_b = kt_tiles[kb]
                    psk = psproj.tile([128, 512], F32, name="psk", tag="proj")
                    for cc in range(NM):
                        mm(psk[:, :],
                           wk_sb[:, cc * DIM + mi * 128: cc * DIM + (mi + 1) * 128],
                           xt_b[:, cc * L:(cc + 1) * L], cc == 0, cc == NM - 1)
                    nc.vector.tensor_copy(kt_b[:, mi * 512:(mi + 1) * 512],
                                          psk[:, :])

                def psv_step(sc, half, kb=kb):
                    xt_b = xt_tiles[kb]
                    if sc == 0 and half == 0:
                        v_b = vbp.tile([128, 4 * H * (D + 1)], BF16,
                                       name="v_b", tag="v")
                        # head h at cols h*65..h*65+63 of each sub-chunk,
                        # col h*65+64 = 1.0 (softmax-sum trick): memset only
                        # those 48 strided columns
                        ones_cols = v_b[:, :].rearrange(
                            "p (x j) -> p x j", j=D + 1)[:, :, D:D + 1]
                        nc.vector.memset(ones_cols, 1.0)
                        vsc_tiles[kb] = [
                            v_b[:, s * H * (D + 1):(s + 1) * H * (D + 1)]
                            for s in range(4)]
                    v_t = vsc_tiles[kb][sc]
                    psv = psproj.tile([128, 512], F32, name="psv", tag="proj")
                    for cc in range(NM):
                        mm(psv[:, 0:384],
                           xt_b[:, cc * L + sc * 128: cc * L + (sc + 1) * 128],
                           wv_sb[:, cc * DIM + half * 384: cc * DIM + (half + 1) * 384],
                           cc == 0, cc == NM - 1)
                    dst = v_t[:, half * 6 * 65:(half + 1) * 6 * 65]
                    dst = dst.rearrange("p (h j) -> p h j", j=65)[:, :, 0:64]
                    srcp = psv[:, 0:384].rearrange("p (h j) -> p h j", j=64)
                    # split the PSUM->SBUF copies between Act and DVE;
                    # exp/copy share one act table so no reload thrash
                    if sc < 2:
                        nc.scalar.copy(dst, srcp)
                    else:
                        nc.vector.tensor_copy(dst, srcp)

                steps = [lambda mi=mi: psk_step(mi) for mi in range(NM)]
                steps += [lambda sc=sc, half=half: psv_step(sc, half)
                          for sc in range(4) for half in range(2)]
                return steps

            def scores_exp(p, h, g):
                # stage 1 of a unit-pair: scores + exp for both key blocks.
                # Returns the context consumed by av_add one slot later, so
                # the Act-engine exps overlap the next unit's PE work.
                po = (h % 2) * 64            # partition offset of head h
                co = (h // 2) * 512          # col offset (mm block h//2)
                qcol = g * NM * LG + (h // 2) * LG
                exps = []
                for kb in (2 * p, 2 * p + 1):
                    kt_b = kt_tiles[kb]
                    mcol = kb if g == 0 else NKB_A + kb
                    ps_s = pssc.tile([128, 1024], F32, name="ps_s", tag="sc")
                    for sc in range(4):
                        mm(ps_s[:, sc * LG:(sc + 1) * LG],
                           kt_b[po:po + 64, co + sc * 128: co + (sc + 1) * 128],
                           qt_sb[po:po + 64, qcol:qcol + LG], True, True)
                    exp_t = expp.tile([128, 1024], BF16, name="exp_t",
                                      tag="exp")
                    nc.scalar.activation(
                        exp_t[:, :], ps_s[:, :],
                        mybir.ActivationFunctionType.Exp,
                        bias=mask_sb[:, mcol:mcol + 1], scale=SCALE,
                    )
                    exps.append(exp_t)
                return (p, h, g, exps)

            def av_add(ctx):
                # stage 2: both key blocks accumulate into ONE [65, 256]
                # PSUM tile (start on the first AV sub-matmul, stop on the
                # last), so a single DVE add per unit-pair updates acc65
                p, h, g, exps = ctx
                ps_o = psav.tile([D + 1, LG], F32, name="ps_o", tag="av")
                for idx, kb in enumerate((2 * p, 2 * p + 1)):
                    for sc in range(4):
                        mm(ps_o[:, :],
                           vsc_tiles[kb][sc][:, h * 65:(h + 1) * 65],
                           exps[idx][:, sc * LG:(sc + 1) * LG],
                           idx == 0 and sc == 0, idx == 1 and sc == 3)
                ucol = (h * 2 + g) * LG
                if p == 0:
                    nc.vector.tensor_copy(acc65[:, ucol:ucol + LG], ps_o[:, :])
                else:
                    nc.vector.tensor_add(acc65[:, ucol:ucol + LG],
                                         acc65[:, ucol:ucol + LG], ps_o[:, :])

            def norm_step(g, mmi):
                # broadcast 1/sum across partitions via outer product with
                # ones; the two heads land in different PSUM banks, then the
                # normalize multiply repacks acc65 into the O-proj layout
                rb_ps = pssc.tile([128, 1024], F32, name="rb_ps", tag="sc")
                for sub in range(2):
                    h = mmi * 2 + sub
                    ucol = (h * 2 + g) * LG
                    stage = outp.tile([1, LG], F32R, name="stage_r", tag="stg")
                    nc.vector.reciprocal(stage[0:1, :],
                                         acc65[64:65, ucol:ucol + LG])
                    mm(rb_ps[0:64, sub * 512:sub * 512 + LG],
                       ones_r[0:1, :], stage[0:1, :], True, True)
                acol = g * NM * LG + mmi * LG
                for sub in range(2):
                    h = mmi * 2 + sub
                    ucol = (h * 2 + g) * LG
                    nc.vector.tensor_mul(
                        at_acc[sub * 64:(sub + 1) * 64, acol:acol + LG],
                        acc65[0:64, ucol:ucol + LG],
                        rb_ps[0:64, sub * 512:sub * 512 + LG])

            def oproj_step(g, mmi):
                pso = psproj.tile([128, L], F32, name="pso", tag="proj")
                for cc in range(NM):
                    mm(pso[:, 0:LG], wo_t[cc][:, mmi * 128:(mmi + 1) * 128],
                       at_acc[:, g * NM * LG + cc * LG: g * NM * LG + (cc + 1) * LG],
                       cc == 0, cc == NM - 1)
                out_t = outp.tile([128, LG], F32, name="out_t", tag="out")
                nc.vector.tensor_scalar_add(out_t[:, :], pso[:, 0:LG],
                                            bo_sb[:, mmi:mmi + 1])
                nc.sync.dma_start(
                    outT[mmi * 128:(mmi + 1) * 128, g * LG:(g + 1) * LG],
                    out_t[:, :])

            def wo_dma_step(cc):
                w = wstr.tile([128, DIM], BF16, name=f"wo_{cc}", tag="wstr")
                nc.gpsimd.dma_start(w[:, :], WoT[cc * 128:(cc + 1) * 128, :])
                wo_t.append(w)

            # ---- prologue: first pair projected up front ----
            dma_xt(0)
            dma_xt(1)
            for st in proj_steps(0) + proj_steps(1):
                st()

            # ---- pair loop, software-pipelined ----
            npair = nkb // 2
            pending = None
            for p in range(npair):
                units = [(h, g) for h in range(H)
                         for g in ([1] if 2 * p >= NKB_A else [1, 0])]
                steps = []
                if p + 1 < npair:
                    steps.append(lambda p=p: dma_xt(2 * p + 2))
                    steps.append(lambda p=p: dma_xt(2 * p + 3))
                    steps += proj_steps(2 * p + 2)
                    steps += proj_steps(2 * p + 3)
                if 2 * p == NKB_A - 2:
                    # load Wo while group A finishes
                    steps += [lambda cc=cc: wo_dma_step(cc)
                              for cc in range(NM)]
                if 2 * p == NKB_A:
                    # group A (g=0) is final after kb 3: normalize it here
                    steps += [lambda mmi=mmi: norm_step(0, mmi)
                              for mmi in range(NM)]
                if 2 * p == NKB_A + 2:
                    steps += [lambda mmi=mmi: oproj_step(0, mmi)
                              for mmi in range(NM)]
                nu, ns = len(units), len(steps)
                si = 0
                for i, (h, g) in enumerate(units):
                    ctx = scores_exp(p, h, g)
                    if pending is not None:
                        av_add(pending)
                    pending = ctx
                    target = (i + 1) * ns // nu
                    while si < target:
                        steps[si]()
                        si += 1
            if pending is not None:
                av_add(pending)
                pending = None

            # ---- tail: group B normalize + output projection ----
            for mmi in range(NM):
                norm_step(1, mmi)
            for mmi in range(NM):
                oproj_step(1, mmi)

    legalize_multiwaits(nc)
    return nc


_program = None


def make_in_maps(x, Wq, Wk, Wv, Wo, bo):
    import ml_dtypes

    bf16 = ml_dtypes.bfloat16
    xf = np.ascontiguousarray(np.asarray(x, np.float32).reshape(S, DIM))
    xT = np.ascontiguousarray(xf.T.astype(bf16))
    shared = {
        "xT": xT,
        "WqT": np.ascontiguousarray(np.asarray(Wq, np.float32).T.astype(bf16)),
        "WkT": np.ascontiguousarray(np.asarray(Wk, np.float32).T.astype(bf16)),
        "WvT": np.ascontiguousarray(np.asarray(Wv, np.float32).T.astype(bf16)),
        "WoT": np.ascontiguousarray(np.asarray(Wo, np.float32).T.astype(bf16)),
        "boT": np.ascontiguousarray(
            np.asarray(bo, np.float32).reshape(NM, 128).T
        ),
    }
    in_maps = []
    for c in range(NC_N):
        hf = c % 2
        va, vb = PAIR_A[c], PAIR_B[c]
        ra = va * 512 + hf * LG
        rb = vb * 512 + hf * LG
        mask = np.zeros((128, NKB_A + NKB), np.float32)
        mask[:, KB_VIEW[va]:NKB_A] = NEG
        mask[:, NKB_A + KB_VIEW[vb]:] = NEG
        m = dict(shared)
        m["xTq"] = np.ascontiguousarray(
            np.concatenate([xT[:, ra:ra + LG], xT[:, rb:rb + LG]], axis=1))
        m["maskT"] = mask
        in_maps.append(m)
    return in_maps


def kernel(x, Wq, Wk, Wv, Wo, bo):
    global _program
    in_maps = make_in_maps(x, Wq, Wk, Wv, Wo, bo)
    if _program is None:
        _program = build_program()
    ret = run_bass_kernel_spmd(_program, in_maps, list(range(NC_N)))
    out = np.empty((S, DIM), np.float32)
    for c in range(NC_N):
        hf = c % 2
        oT = ret.results[c]["outT"]
        ra = PAIR_A[c] * 512 + hf * LG
        rb = PAIR_B[c] * 512 + hf * LG
        out[ra:ra + LG, :] = oT[:, 0:LG].T
        out[rb:rb + LG, :] = oT[:, LG:2 * LG].T
    return out.reshape(1, V, L, DIM)


# revision 20
# speedup vs baseline: 215.6741x; 1.0685x over previous
"""Block-sparse attention kernel for TRN2 (8 NeuronCores, SPMD).

Math (from the reference nn.Module):
  x [1, 8, 512, 768] -> flatten to [S=4096, 768]
  q/k/v = x @ W{q,k,v}.T, split into H=12 heads of D=64
  block mask: query in view v attends keys [0 : P_v] where
  P_v = 1024 for v in {0,1}, (v+1)*512 for v >= 2  (always a prefix)
  out = softmax(q k^T / 8 + mask) v, merge heads, @ Wo.T + bo

Sharding: core c owns two 256-query groups (group A from a low view,
group B from a high view; see PAIR_A/PAIR_B). Every core computes the
full K/V projections (replicated; avoids collectives) and masks keys
beyond its prefix via an exp-bias of -1e5 per 512-key block.

Dataflow is fully transposed (x^T, K^T, Q^T, out^T) so every matmul has
its contraction dim on partitions; projections run f32r with a >=256
moving operand (full PE rate), attention (Q^T, K^T, V, exp) runs bf16
(also full rate, ~4e-3 rel err, well inside the 2e-2 budget). Softmax
sums come free from a ones column interleaved into the V tiles (head h
at columns h*65..+64, column h*65+64 is 1.0): the AV matmul's 65th
output row is the per-query sum of exp.

Schedule: key blocks are processed in PAIRS. Per (head, group)
unit-pair, the two AV matmul groups accumulate into one [65, 256] PSUM
tile (values + sum row together), so only ONE DVE add per unit-pair
lands in the 65-row accumulator. The K/V projection of the next pair
plus the group-A normalize/output-projection (final after kb 3) are
interleaved between attention units so the PE never idles (TRN2's
p-state ramp halves PE clock after every stall). exp runs on Act (the
only engine with activation); V-block PSUM->SBUF copies are split
Act/DVE; the score PSUM pool is double-buffered so exp overlaps the
next scores matmul.
"""

import sys

sys.path.insert(0, "/opt/trn_rl_repo")

import numpy as np

import concourse.bass as bass
import concourse.mybir as mybir
import concourse.tile as tile
from concourse.bass_utils import run_bass_kernel_spmd

F32 = mybir.dt.float32
F32R = mybir.dt.float32r
BF16 = mybir.dt.bfloat16

S, DIM, H, D = 4096, 768, 12, 64
V, L = 8, 512
NC_N = 8
NM = DIM // 128          # 6 chunks of the model dim
NKB = S // 512           # 8 key blocks
SCALE = float(D) ** -0.5
NEG = -1.0e5

# allowed 512-key blocks per view (prefix length / 512)
KB_VIEW = [2, 2, 3, 4, 5, 6, 7, 8]
# core c handles half (c%2) of views PAIR_A[c] (group A, compiled 4 kb)
# and PAIR_B[c] (group B, compiled 8 kb), 256 queries each
PAIR_A = [0, 0, 1, 1, 2, 2, 3, 3]
PAIR_B = [7, 7, 6, 6, 5, 5, 4, 4]
NKB_A = 4
LG = 256  # queries per group


def legalize_multiwaits(nc):
    """This toolchain's walrus accepts at most ONE sync-wait per
    instruction; Tile's sem-assignment happily emits several. Split the
    extras into standalone EventSemaphore (wait) instructions on the same
    engine, placed immediately before the gated instruction."""
    scratch = nc.alloc_semaphore("legalize_scratch")
    fn = nc.m.functions[0]
    for bb in fn.blocks:
        insts = bb.instructions
        out = []
        changed = False
        for inst in insts:
            si = getattr(inst, "sync_info", None)
            ow = list(si.on_wait) if si is not None and si.on_wait else []
            if len(ow) > 1:
                for w in ow[:-1]:
                    ev = nc.engines[inst.engine].nop(nofuse=True)
                    raw = ev.ins
                    raw.sync_info = mybir.SyncInfo(on_wait=[w], on_update=[])
                    # pop it from wherever the builder appended it
                    tail = nc.cur_bb.bb.instructions
                    assert tail[-1].name == raw.name
                    nc.cur_bb.bb.instructions = tail[:-1]
                    out.append(raw)
                si.on_wait = [ow[-1]]
                inst.sync_info = si
                changed = True
            out.append(inst)
        if changed:
            bb.instructions = out


def build_program(nkb=NKB, loop_n=1):
    assert nkb % 2 == 0, "key blocks are processed in pairs"
    nc = bass.Bass()
    xT = nc.dram_tensor("xT", [DIM, S], BF16, kind="ExternalInput")
    xTq = nc.dram_tensor("xTq", [DIM, L], BF16, kind="ExternalInput")
    WqT = nc.dram_tensor("WqT", [DIM, DIM], BF16, kind="ExternalInput")
    WkT = nc.dram_tensor("WkT", [DIM, DIM], BF16, kind="ExternalInput")
    WvT = nc.dram_tensor("WvT", [DIM, DIM], BF16, kind="ExternalInput")
    WoT = nc.dram_tensor("WoT", [DIM, DIM], BF16, kind="ExternalInput")
    boT = nc.dram_tensor("boT", [128, NM], F32, kind="ExternalInput")
    maskT = nc.dram_tensor("maskT", [128, NKB_A + NKB], F32, kind="ExternalInput")
    outT = nc.dram_tensor("outT", [DIM, L], F32, kind="ExternalOutput")

    def mm(out, lhsT, rhs, start, stop):
        nc.tensor.matmul(out, lhsT, rhs, start=start, stop=stop)

    with nc.allow_low_precision(reason="bf16 attention operands, ~4e-3"), \
         tile.TileContext(nc) as tc, \
         tc.tile_pool(name="const", bufs=1) as cpool, \
         tc.tile_pool(name="wres", bufs=1) as wres, \
         tc.tile_pool(name="wstream", bufs=6) as wstr, \
         tc.tile_pool(name="acc", bufs=1) as accp, \
         tc.tile_pool(name="xt", bufs=3) as xtp, \
         tc.tile_pool(name="ktblk", bufs=4) as ktp, \
         tc.tile_pool(name="vblk", bufs=4) as vbp, \
         tc.tile_pool(name="expp", bufs=4) as expp, \
         tc.tile_pool(name="outp", bufs=2) as outp, \
         tc.tile_pool(name="ps_proj", bufs=2, space="PSUM") as psproj, \
         tc.tile_pool(name="ps_sc", bufs=2, space="PSUM") as pssc, \
         tc.tile_pool(name="ps_av", bufs=2, space="PSUM") as psav:

        mask_sb = cpool.tile([128, NKB_A + NKB], F32, name="mask_sb")
        nc.sync.dma_start(mask_sb[:, :], maskT[:, :])
        ones_r = cpool.tile([65, 64], F32R, name="ones_r")
        nc.vector.memset(ones_r[:, :].bitcast(F32), 1.0)
        bo_sb = cpool.tile([128, NM], F32, name="bo_sb")
        nc.sync.dma_start(bo_sb[:, :], boT[:, :])

        # resident K/V weights: block cc at cols cc*DIM, rows = W*T rows
        wk_sb = wres.tile([128, NM * DIM], BF16, name="wk_sb")
        wv_sb = wres.tile([128, NM * DIM], BF16, name="wv_sb")
        # SP ring: keeps the bulk K/V weight load off the gpsimd ring so
        # the Q-projection operands (xq, wq) land first
        for cc in range(NM):
            nc.sync.dma_start(
                wk_sb[:, cc * DIM:(cc + 1) * DIM], WkT[cc * 128:(cc + 1) * 128, :]
            )
            nc.sync.dma_start(
                wv_sb[:, cc * DIM:(cc + 1) * DIM], WvT[cc * 128:(cc + 1) * 128, :]
            )

        for _rep in range(loop_n):
            # ---- Q projection: Q^T[mm-block] = sum_cc WqT[cc,mm].T @ xTq[cc] ----
            # cols [g*NM*LG + mi*LG : +LG] = group g, m-chunk mi
            qt_sb = accp.tile([128, 2 * NM * LG], BF16, name="qt_sb")
            xq_sb = xtp.tile([128, NM * L], BF16, name="xt_t", tag="xt")
            for cc in range(NM):
                nc.gpsimd.dma_start(
                    xq_sb[:, cc * L:(cc + 1) * L], xTq[cc * 128:(cc + 1) * 128, :]
                )
            wq_t = []
            for cc in range(NM):
                w = wstr.tile([128, DIM], BF16, name=f"wq_{cc}", tag="wstr")
                nc.gpsimd.dma_start(w[:, :], WqT[cc * 128:(cc + 1) * 128, :])
                wq_t.append(w)
            for mi in range(NM):
                psq = psproj.tile([128, L], F32, name="psq", tag="proj")
                for cc in range(NM):
                    mm(psq[:, :], wq_t[cc][:, mi * 128:(mi + 1) * 128],
                       xq_sb[:, cc * L:(cc + 1) * L], cc == 0, cc == NM - 1)
                # xTq is [A queries 0:256 | B queries 256:512]
                for g in range(2):
                    nc.vector.tensor_copy(
                        qt_sb[:, g * NM * LG + mi * LG: g * NM * LG + (mi + 1) * LG],
                        psq[:, g * LG:(g + 1) * LG])

            # persistent 65-row accumulator: rows 0..63 = AV values, row 64 =
            # sum of exp. unit (h, g) lives at cols (h*2+g)*LG.
            acc65 = accp.tile([65, 2 * H * LG], F32, name="acc65")
            # at_acc: O-proj moving operand layout [128 (2 heads), NM*LG] per
            # group; built by fused repack+normalize from acc65
            at_acc = accp.tile([128, 2 * NM * LG], BF16, name="at_acc")

            # per-kb tile state for the software pipeline
            xt_tiles, kt_tiles, vsc_tiles = {}, {}, {}
            wo_t = []

            def dma_xt(kb):
                xt_b = xtp.tile([128, NM * L], BF16, name="xt_t", tag="xt")
                for cc in range(NM):
                    nc.gpsimd.dma_start(
                        xt_b[:, cc * L:(cc + 1) * L],
                        xT[cc * 128:(cc + 1) * 128, kb * 512:(kb + 1) * 512],
                    )
                xt_tiles[kb] = xt_b

            def proj_steps(kb):
                # 14 closures: 6 K^T m-chunks + 8 V (sc, half) chunks,
                # each one PSUM group — interleaved between attention units
                # of the previous pair so PE never idles
                def psk_step(mi, kb=kb):
                    xt_b = xt_tiles[kb]
                    if mi == 0:
                        kt_tiles[kb] = ktp.tile([128, NM * 512], BF16,
                                                name="kt_b", tag="kt")
                    kt_b = kt_tiles[kb]
                    psk = psproj.tile([128, 512], F32, name="psk", tag="proj")
                    for cc in range(NM):
                        mm(psk[:, :],
                           wk_sb[:, cc * DIM + mi * 128: cc * DIM + (mi + 1) * 128],
                           xt_b[:, cc * L:(cc + 1) * L], cc == 0, cc == NM - 1)
                    nc.vector.tensor_copy(kt_b[:, mi * 512:(mi + 1) * 512],
                                          psk[:, :])

                def psv_step(sc, half, kb=kb):
                    xt_b = xt_tiles[kb]
                    if sc == 0 and half == 0:
                        v_b = vbp.tile([128, 4 * H * (D + 1)], BF16,
                                       name="v_b", tag="v")
                        # head h at cols h*65..h*65+63 of each sub-chunk,
                        # col h*65+64 = 1.0 (softmax-sum trick): memset only
                        # those 48 strided columns
                        ones_cols = v_b[:, :].rearrange(
                            "p (x j) -> p x j", j=D + 1)[:, :, D:D + 1]
                        nc.vector.memset(ones_cols, 1.0)
                        vsc_tiles[kb] = [
                            v_b[:, s * H * (D + 1):(s + 1) * H * (D + 1)]
                            for s in range(4)]
                    v_t = vsc_tiles[kb][sc]
                    psv = psproj.tile([128, 512], F32, name="psv", tag="proj")
                    for cc in range(NM):
                        mm(psv[:, 0:384],
                           xt_b[:, cc * L + sc * 128: cc * L + (sc + 1) * 128],
                           wv_sb[:, cc * DIM + half * 384: cc * DIM + (half + 1) * 384],
                           cc == 0, cc == NM - 1)
                    dst = v_t[:, half * 6 * 65:(half + 1) * 6 * 65]
                    dst = dst.rearrange("p (h j) -> p h j", j=65)[:, :, 0:64]
                    srcp = psv[:, 0:384].rearrange("p (h j) -> p h j", j=64)
                    # split the PSUM->SBUF copies between Act and DVE;
                    # exp/copy share one act table so no reload thrash
                    if sc < 2:
                        nc.scalar.copy(dst, srcp)
                    else:
                        nc.vector.tensor_copy(dst, srcp)

                steps = [lambda mi=mi: psk_step(mi) for mi in range(NM)]
                steps += [lambda sc=sc, half=half: psv_step(sc, half)
                          for sc in range(4) for half in range(2)]
                return steps

            def scores_exp(p, h, g):
                # stage 1 of a unit-pair: scores + exp for both key blocks.
                # Returns the context consumed by av_add one slot later, so
                # the Act-engine exps overlap the next unit's PE work.
                po = (h % 2) * 64            # partition offset of head h
                co = (h // 2) * 512          # col offset (mm block h//2)
                qcol = g * NM * LG + (h // 2) * LG
                exps = []
                for kb in (2 * p, 2 * p + 1):
                    kt_b = kt_tiles[kb]
                    mcol = kb if g == 0 else NKB_A + kb
                    ps_s = pssc.tile([128, 1024], F32, name="ps_s", tag="sc")
                    for sc in range(4):
                        mm(ps_s[:, sc * LG:(sc + 1) * LG],
                           kt_b[po:po + 64, co + sc * 128: co + (sc + 1) * 128],
                           qt_sb[po:po + 64, qcol:qcol + LG], True, True)
                    exp_t = expp.tile([128, 1024], BF16, name="exp_t",
                                      tag="exp")
                    nc.scalar.activation(
                        exp_t[:, :], ps_s[:, :],
                        mybir.ActivationFunctionType.Exp,
                        bias=mask_sb[:, mcol:mcol + 1], scale=SCALE,
                    )
                    exps.append(exp_t)
                return (p, h, g, exps)

            def av_add(ctx):
                # stage 2: both key blocks accumulate into ONE [65, 256]
                # PSUM tile (start on the first AV sub-matmul, stop on the
                # last), so a single DVE add per unit-pair updates acc65
                p, h, g, exps = ctx
                ps_o = psav.tile([D + 1, LG], F32, name="ps_o", tag="av")
                for idx, kb in enumerate((2 * p, 2 * p + 1)):
                    for sc in range(4):
                        mm(ps_o[:, :],
                           vsc_tiles[kb][sc][:, h * 65:(h + 1) * 65],
                           exps[idx][:, sc * LG:(sc + 1) * LG],
                           idx == 0 and sc == 0, idx == 1 and sc == 3)
                ucol = (h * 2 + g) * LG
                if p == 0:
                    nc.vector.tensor_copy(acc65[:, ucol:ucol + LG], ps_o[:, :])
                else:
                    nc.vector.tensor_add(acc65[:, ucol:ucol + LG],
                                         acc65[:, ucol:ucol + LG], ps_o[:, :])

            def norm_step(g, mmi):
                # broadcast 1/sum across partitions via outer product with
                # ones; the two heads land in different PSUM banks, then the
                # normalize multiply repacks acc65 into the O-proj layout
                rb_ps = pssc.tile([128, 1024], F32, name="rb_ps", tag="sc")
                for sub in range(2):
                    h = mmi * 2 + sub
                    ucol = (h * 2 + g) * LG
                    stage = outp.tile([1, LG], F32R, name="stage_r", tag="stg")
                    nc.vector.reciprocal(stage[0:1, :],
                                         acc65[64:65, ucol:ucol + LG])
                    mm(rb_ps[0:64, sub * 512:sub * 512 + LG],
                       ones_r[0:1, :], stage[0:1, :], True, True)
                acol = g * NM * LG + mmi * LG
                for sub in range(2):
                    h = mmi * 2 + sub
                    ucol = (h * 2 + g) * LG
                    nc.vector.tensor_mul(
                        at_acc[sub * 64:(sub + 1) * 64, acol:acol + LG],
                        acc65[0:64, ucol:ucol + LG],
                        rb_ps[0:64, sub * 512:sub * 512 + LG])

            def oproj_step(g, mmi):
                pso = psproj.tile([128, L], F32, name="pso", tag="proj")
                for cc in range(NM):
                    mm(pso[:, 0:LG], wo_t[cc][:, mmi * 128:(mmi + 1) * 128],
                       at_acc[:, g * NM * LG + cc * LG: g * NM * LG + (cc + 1) * LG],
                       cc == 0, cc == NM - 1)
                out_t = outp.tile([128, LG], F32, name="out_t", tag="out")
                nc.vector.tensor_scalar_add(out_t[:, :], pso[:, 0:LG],
                                            bo_sb[:, mmi:mmi + 1])
                nc.sync.dma_start(
                    outT[mmi * 128:(mmi + 1) * 128, g * LG:(g + 1) * LG],
                    out_t[:, :])

            def wo_dma_step(cc):
                w = wstr.tile([128, DIM], BF16, name=f"wo_{cc}", tag="wstr")
                nc.gpsimd.dma_start(w[:, :], WoT[cc * 128:(cc + 1) * 128, :])
                wo_t.append(w)

            # ---- prologue: first pair projected up front ----
            dma_xt(0)
            dma_xt(1)
            for st in proj_steps(0) + proj_steps(1):
                st()

            # ---- pair loop, software-pipelined ----
            npair = nkb // 2
            pending = None
            for p in range(npair):
                units = [(h, g) for h in range(H)
                         for g in ([1] if 2 * p >= NKB_A else [1, 0])]
                steps = []
                if p + 1 < npair:
                    steps.append(lambda p=p: dma_xt(2 * p + 2))
                    steps.append(lambda p=p: dma_xt(2 * p + 3))
                    steps += proj_steps(2 * p + 2)
                    steps += proj_steps(2 * p + 3)
                if 2 * p == NKB_A - 2:
                    # load Wo while group A finishes
                    steps += [lambda cc=cc: wo_dma_step(cc)
                              for cc in range(NM)]
                if 2 * p == NKB_A:
                    # group A (g=0) is final after kb 3: normalize it here
                    steps += [lambda mmi=mmi: norm_step(0, mmi)
                              for mmi in range(NM)]
                if 2 * p == NKB_A + 2:
                    steps += [lambda mmi=mmi: oproj_step(0, mmi)
                              for mmi in range(NM)]
                nu, ns = len(units), len(steps)
                si = 0
                for i, (h, g) in enumerate(units):
                    ctx = scores_exp(p, h, g)
                    if pending is not None:
                        av_add(pending)
                    pending = ctx
                    target = (i + 1) * ns // nu
                    while si < target:
                        steps[si]()
                        si += 1
            if pending is not None:
                av_add(pending)
                pending = None

            # ---- tail: group B normalize + output projection ----
            for mmi in range(NM):
                norm_step(1, mmi)
            for mmi in range(NM):
                oproj_step(1, mmi)

    legalize_multiwaits(nc)
    return nc


_program = None


def make_in_maps(x, Wq, Wk, Wv, Wo, bo):
    import ml_dtypes

    bf16 = ml_dtypes.bfloat16
    xf = np.ascontiguousarray(np.asarray(x, np.float32).reshape(S, DIM))
    xT = np.ascontiguousarray(xf.T.astype(bf16))
    shared = {
        "xT": xT,
        "WqT": np.ascontiguousarray(np.asarray(Wq, np.float32).T.astype(bf16)),
        "WkT": np.ascontiguousarray(np.asarray(Wk, np.float32).T.astype(bf16)),
        "WvT": np.ascontiguousarray(np.asarray(Wv, np.float32).T.astype(bf16)),
        "WoT": np.ascontiguousarray(np.asarray(Wo, np.float32).T.astype(bf16)),
        "boT": np.ascontiguousarray(
            np.asarray(bo, np.float32).reshape(NM, 128).T
        ),
    }
    in_maps = []
    for c in range(NC_N):
        hf = c % 2
        va, vb = PAIR_A[c], PAIR_B[c]
        ra = va * 512 + hf * LG
        rb = vb * 512 + hf * LG
        mask = np.zeros((128, NKB_A + NKB), np.float32)
        mask[:, KB_VIEW[va]:NKB_A] = NEG
        mask[:, NKB_A + KB_VIEW[vb]:] = NEG
        m = dict(shared)
        m["xTq"] = np.ascontiguousarray(
            np.concatenate([xT[:, ra:ra + LG], xT[:, rb:rb + LG]], axis=1))
        m["maskT"] = mask
        in_maps.append(m)
    return in_maps


def kernel(x, Wq, Wk, Wv, Wo, bo):
    global _program
    in_maps = make_in_maps(x, Wq, Wk, Wv, Wo, bo)
    if _program is None:
        _program = build_program()
    ret = run_bass_kernel_spmd(_program, in_maps, list(range(NC_N)))
    out = np.empty((S, DIM), np.float32)
    for c in range(NC_N):
        hf = c % 2
        oT = ret.results[c]["outT"]
        ra = PAIR_A[c] * 512 + hf * LG
        rb = PAIR_B[c] * 512 + hf * LG
        out[ra:ra + LG, :] = oT[:, 0:LG].T
        out[rb:rb + LG, :] = oT[:, LG:2 * LG].T
    return out.reshape(1, V, L, DIM)
